# revision 1
# baseline (speedup 1.0000x reference)
"""Trainium2 Bass kernel for nn_CustomMultiHeadAttention_20418274525443.

Self-contained: takes FULL unsharded inputs (as produced by the problem's
setup_inputs), shards across 8 NeuronCores, runs a Bass/Tile kernel via
run_bass_kernel_spmd, and gathers the full output.

Sharding: core c handles batch b = c//4 and heads 4*(c%4) .. 4*(c%4)+3
(data parallel on B x tensor parallel on heads). Each core computes its
partial output projection (contribution of its 256 hidden dims); the host
sums the 4 partials per batch and adds the output bias.

Math per core (Dh = 64, scale = 1/8):
  Q^T = (Wq_slice @ x_q^T) + bq    [256, 2048]  (d-on-partitions layout)
  K^T likewise; V = x_v @ Wv_slice^T + bv  [2048, 256] (natural layout)
  per head h: scores^T[k,q] = K_h^T.T @ Q_h^T  (PSUM, preloaded with M')
    M'[k,q] = 8*alpha*copysign(log1p|f_q - f_k|, f_q - f_k) (+8*attn_mask^T)
    probs^T = Exp(0.125 * PSUM + kpm_bias_k)  -> bf16
    outext^T[dv+1, q] = [V_h | 1]^T.T-style: lhsT=[V_h|ones] so row 64 = sum_k probs
    scaled^T = outext^T[0:64] * (1 / outext^T[64])   (per-q denominators)
  out_partial[q, :] = concat_h(scaled^T).T @ Wo_slice^T  [2048, 1024] fp32
"""
import sys

sys.path.insert(0, "/opt/trn_rl_repo")

import numpy as np
import ml_dtypes

import concourse.bass as bass
import concourse.tile as tile
from concourse import bacc, mybir
from concourse.bass_utils import run_bass_kernel_spmd
from concourse.masks import make_identity

D_MODEL = 1024
NHEAD = 16
HEAD_DIM = 64
B, T = 2, 2048
N_CORES = 8
HPC = 4               # heads per core
DSL = HPC * HEAD_DIM  # 256 = per-core slice of d_model
SCALE = HEAD_DIM ** -0.5  # 0.125
QH = 2                # q halves
QHW = T // QH         # 1024
KT = T // 128         # 16 k tiles
F32 = mybir.dt.float32
F16 = mybir.dt.float16
BF16 = mybir.dt.bfloat16
AF = mybir.ActivationFunctionType
ALU = mybir.AluOpType

_PROGRAM_CACHE = {}


def _build_program(use_attn_mask: bool, repeat: int = 1, limit: int = 99):
    # limit: 1=consts+weights, 2=+projections, 3=+M', 4=+scores/exp,
    # 5=+PV/normalize, 99=full
    nc = bacc.Bacc(num_devices=N_CORES)

    # ---- per-core DRAM inputs (host pre-sliced / transposed / cast) ----
    xq_t = nc.dram_tensor("xq_t", [D_MODEL, T], F16, kind="ExternalInput")
    xk_t = nc.dram_tensor("xk_t", [D_MODEL, T], F16, kind="ExternalInput")
    xv_t = nc.dram_tensor("xv_t", [D_MODEL, T], F16, kind="ExternalInput")
    wq_t = nc.dram_tensor("wq_t", [D_MODEL, DSL], F16, kind="ExternalInput")
    wk_t = nc.dram_tensor("wk_t", [D_MODEL, DSL], F16, kind="ExternalInput")
    wv_t = nc.dram_tensor("wv_t", [D_MODEL, DSL], F16, kind="ExternalInput")
    wo_t = nc.dram_tensor("wo_t", [DSL, D_MODEL], F16, kind="ExternalInput")
    # host-prearranged constants: [128, .] layouts so every DMA is a plain
    # contiguous per-partition transfer (strided/stride-0 descriptor patterns
    # are pathologically slow)
    bqc_d = nc.dram_tensor("bqc_d", [128, 2], F32, kind="ExternalInput")
    bkc_d = nc.dram_tensor("bkc_d", [128, 2], F32, kind="ExternalInput")
    bvb_d = nc.dram_tensor("bvb_d", [128, DSL], F32, kind="ExternalInput")
    fqb_d = nc.dram_tensor("fqb_d", [128, T], F32, kind="ExternalInput")
    fcol_d = nc.dram_tensor("fcol_d", [128, KT], F32, kind="ExternalInput")
    kb_d = nc.dram_tensor("kb_d", [128, KT], F32, kind="ExternalInput")
    alc_d = nc.dram_tensor("alc_d", [128, 2], F32, kind="ExternalInput")
    if use_attn_mask:
        am_d = nc.dram_tensor("am_d", [T, T], F16, kind="ExternalInput")
    out_d = nc.dram_tensor("out_d", [T, D_MODEL], F32, kind="ExternalOutput")

    def bcast_ap(src_ap, parts):
        # partition-broadcast of a 1-row / 1-d tensor across `parts` partitions
        return bass.AP(tensor=src_ap.tensor, offset=src_ap.offset,
                       ap=[[0, parts]] + list(src_ap.ap[-1:]))

    def mm(out_ap, lhsT, rhs, start, stop, nmax=512):
        # matmul with the free dim chunked to one PSUM bank (<=512 fp32)
        n = rhs.shape[-1]
        assert out_ap.shape[-1] == n
        for c0 in range(0, n, nmax):
            c = slice(c0, min(c0 + nmax, n))
            nc.tensor.matmul(out_ap[:, c], lhsT, rhs[:, c],
                             start=start, stop=stop)

    with tile.TileContext(nc) as tc:
        import contextlib
        with contextlib.ExitStack() as ctx:
            # NOTE: pool `bufs` is per-tag; tiles with distinct names/tags each
            # get their own `bufs` slots.
            const = ctx.enter_context(tc.tile_pool(name="const", bufs=1))
            qk_pool = ctx.enter_context(tc.tile_pool(name="qk", bufs=1))
            v_pool = ctx.enter_context(tc.tile_pool(name="vsb", bufs=1))
            w2_pool = ctx.enter_context(tc.tile_pool(name="wo", bufs=1))
            mp_pool = ctx.enter_context(tc.tile_pool(name="mp", bufs=KT))
            tmp_pool = ctx.enter_context(tc.tile_pool(name="tmp", bufs=3))
            probs_pool = ctx.enter_context(tc.tile_pool(name="probs", bufs=3))
            den_pool = ctx.enter_context(tc.tile_pool(name="den", bufs=2))
            opair_pool = ctx.enter_context(tc.tile_pool(name="opair", bufs=4))
            ostage_pool = ctx.enter_context(tc.tile_pool(name="ostage", bufs=3))

            # ---- constants ----
            # identity scaled by 8*alpha: the M'-preload matmul I'.T @ M'
            # then contributes 8*alpha*copysign(log1p|df|, df) to the scores
            # PSUM, so M' itself stays a pure +-log1p and alpha stays a
            # runtime input (no recompile on alpha change).
            ident_f = const.tile([128, 128], F32)
            make_identity(nc, ident_f[:, :])
            ident = const.tile([128, 128], F16)
            ones64 = const.tile([1, 64], F32)
            nc.vector.memset(ones64[:, :], 1.0)

            fq_bc = const.tile([128, T], F32)
            nc.sync.dma_start(out=fq_bc[:, :], in_=fqb_d.ap())
            fk_col = const.tile([128, KT], F32)
            nc.sync.dma_start(out=fk_col[:, :], in_=fcol_d.ap())
            kbias = const.tile([128, KT], F32)
            nc.sync.dma_start(out=kbias[:, :], in_=kb_d.ap())
            al_col = const.tile([128, 2], F32)
            nc.sync.dma_start(out=al_col[:, :], in_=alc_d.ap())
            nc.vector.tensor_scalar(ident[:, :], ident_f[:, :],
                                    al_col[:, 0:1], None, op0=ALU.mult)
            bq_col = const.tile([128, 2], F32)
            nc.sync.dma_start(out=bq_col[:, :], in_=bqc_d.ap())
            bk_col = const.tile([128, 2], F32)
            nc.sync.dma_start(out=bk_col[:, :], in_=bkc_d.ap())
            bv_bc = const.tile([128, DSL], F32)
            nc.sync.dma_start(out=bv_bc[:, :], in_=bvb_d.ap())

            # ---- weights ----
            w_sb = {}
            for nm, dram in (("q", wq_t), ("k", wk_t), ("v", wv_t)):
                for di in range(8):
                    t_ = const.tile([128, DSL], F16, name=f"w{nm}{di}",
                                    tag=f"w{nm}{di}")
                    nc.sync.dma_start(out=t_[:, :],
                                      in_=dram.ap()[di * 128:(di + 1) * 128, :])
                    w_sb[nm, di] = t_
            wo_sb = []
            for pr in range(2):
                t_ = w2_pool.tile([128, D_MODEL], F16, name=f"wo{pr}")
                nc.sync.dma_start(out=t_[:, :],
                                  in_=wo_t.ap()[pr * 128:(pr + 1) * 128, :])
                wo_sb.append(t_)

            if use_attn_mask:
                am_sb = []
                for kt in range(KT):
                    t_ = const.tile([128, T], F16, name=f"am{kt}", tag=f"am{kt}")
                    # host passes 8 * attn_mask^T, so [k, q] orientation
                    nc.sync.dma_start(out=t_[:, :],
                                      in_=am_d.ap()[kt * 128:(kt + 1) * 128, :])
                    am_sb.append(t_)

            for _rep in range(repeat):
                # ---- phase 1: projections ----
                qt_sb, kt_sb = [], []
                for i in range(2):
                    qt_sb.append(qk_pool.tile([128, T], F16, name=f"qt{i}"))
                    kt_sb.append(qk_pool.tile([128, T], F16, name=f"kt{i}"))
                v_sb = []
                for kt in range(KT):
                    t_ = v_pool.tile([128, HPC * 65], F16, name=f"v{kt}")
                    nc.vector.memset(t_[:, :], 1.0)  # ones columns survive at 65k+64
                    v_sb.append(t_)

                if limit < 2:
                    continue
                with tc.tile_pool(name="xt", bufs=9) as xt_pool, \
                     tc.tile_pool(name="psA", bufs=2, space="PSUM") as psA:
                    for nm, xdram, bias_col, outs in (
                            ("q", xq_t, bq_col, qt_sb), ("k", xk_t, bk_col, kt_sb)):
                        x_tiles = []
                        for di in range(8):
                            xt_ = xt_pool.tile([128, T], F16, name=f"x{nm}{di}",
                                               tag="xt")
                            nc.sync.dma_start(
                                out=xt_[:, :],
                                in_=xdram.ap()[di * 128:(di + 1) * 128, :])
                            x_tiles.append(xt_)
                        for do_t in range(2):
                            for nch in range(2):
                                ps = psA.tile([128, QHW], F32, tag="psA")
                                for di in range(8):
                                    mm(ps[:, :],
                                       w_sb[nm, di][:, do_t * 128:(do_t + 1) * 128],
                                       x_tiles[di][:, nch * QHW:(nch + 1) * QHW],
                                       start=(di == 0), stop=(di == 7))
                                # PSUM -> SBUF bf16 with per-partition bias add
                                nc.vector.tensor_scalar(
                                    outs[do_t][:, nch * QHW:(nch + 1) * QHW],
                                    ps[:, :], bias_col[:, do_t:do_t + 1], None,
                                    op0=ALU.add)
                    # V projection (natural layout)
                    x_tiles = []
                    for di in range(8):
                        xt_ = xt_pool.tile([128, T], F16, name=f"xv{di}", tag="xt")
                        nc.sync.dma_start(out=xt_[:, :],
                                          in_=xv_t.ap()[di * 128:(di + 1) * 128, :])
                        x_tiles.append(xt_)
                    for tt in range(KT):
                        ps = psA.tile([128, DSL], F32, tag="psA")
                        for di in range(8):
                            mm(ps[:, :],
                               x_tiles[di][:, tt * 128:(tt + 1) * 128],
                               w_sb["v", di][:, :],
                               start=(di == 0), stop=(di == 7))
                        # strided copy into cols h*65..h*65+63 with bv add; the
                        # ones columns at h*65+64 remain from the memset
                        vdst = v_sb[tt][:, :].rearrange(
                            "p (h e) -> p h e", e=65)[:, :, 0:HEAD_DIM]
                        nc.vector.tensor_tensor(
                            vdst,
                            ps[:, :].rearrange("p (h e) -> p h e", e=HEAD_DIM),
                            bv_bc[:, :].rearrange("p (h e) -> p h e", e=HEAD_DIM),
                            op=ALU.add)

                # ---- phase 2: attention + output proj, per q-half ----
                # PSUM budget (8 banks): psS tag (scores / denom-bcast / O-proj
                # share 2 slots x 2 banks = 4) + psO (2 slots x 2 banks = 4).
                with tc.tile_pool(name="psS", bufs=2, space="PSUM") as psS, \
                     tc.tile_pool(name="psO", bufs=2, space="PSUM") as psO:
                    if limit < 3:
                        continue
                    for qh in range(QH):
                        qsl = slice(qh * QHW, (qh + 1) * QHW)
                        # M' tiles for this q half
                        mp_tiles = []
                        for kt in range(KT):
                            d_t = tmp_pool.tile([128, QHW], F16, name="d_t", tag="d")
                            nc.vector.tensor_scalar(
                                d_t[:, :], fq_bc[:, qsl], fk_col[:, kt:kt + 1], None,
                                op0=ALU.subtract)
                            ge_t = tmp_pool.tile([128, QHW], F16, name="ge_t",
                                                 tag="ge")
                            nc.vector.tensor_scalar(
                                ge_t[:, :], d_t[:, :], 0.0, None, op0=ALU.is_ge)
                            sg_t = tmp_pool.tile([128, QHW], F16, name="sg_t",
                                                 tag="sg")
                            # ge*2 - 1  ->  +-1 (alpha lives in the scaled identity)
                            nc.vector.tensor_scalar(
                                sg_t[:, :], ge_t[:, :], 2.0, -1.0,
                                op0=ALU.mult, op1=ALU.add)
                            a_t = tmp_pool.tile([128, QHW], F16, name="a_t", tag="a")
                            nc.vector.tensor_tensor(a_t[:, :], d_t[:, :], sg_t[:, :],
                                                    op=ALU.mult)  # |d|
                            g_t = tmp_pool.tile([128, QHW], F16, name="g_t", tag="g")
                            nc.scalar.activation(g_t[:, :], a_t[:, :], AF.Ln,
                                                 bias=1.0, scale=1.0)
                            mp = mp_pool.tile([128, QHW], F16, name="mp", tag="mp")
                            if use_attn_mask:
                                nc.vector.scalar_tensor_tensor(
                                    mp[:, :], g_t[:, :], 1.0, sg_t[:, :],
                                    op0=ALU.bypass, op1=ALU.mult)
                                nc.vector.tensor_tensor(
                                    mp[:, :], mp[:, :], am_sb[kt][:, qsl],
                                    op=ALU.add)
                            else:
                                nc.vector.tensor_tensor(mp[:, :], g_t[:, :],
                                                        sg_t[:, :], op=ALU.mult)
                            mp_tiles.append(mp)

                        if limit < 4:
                            continue
                        opairs = []
                        for h in range(HPC):
                            pr_i = h // 2
                            hh = h % 2
                            if hh == 0:
                                op_t = opair_pool.tile([128, QHW], F16,
                                                       name=f"opair{pr_i}",
                                                       tag="opair")
                                opairs.append(op_t)
                            ot = psO.tile([65, QHW], F32, tag="psO")
                            for kt in range(KT):
                                sc = psS.tile([128, QHW], F32, tag="psS")
                                mm(sc[:, :], ident[:, :], mp_tiles[kt][:, :],
                                   start=True, stop=False)
                                mm(sc[:, :],
                                   kt_sb[pr_i][hh * 64:(hh + 1) * 64,
                                               kt * 128:(kt + 1) * 128],
                                   qt_sb[pr_i][hh * 64:(hh + 1) * 64, qsl],
                                   start=False, stop=True)
                                pr = probs_pool.tile([128, QHW], F16, name="pr",
                                                     tag="pr")
                                nc.scalar.activation(pr[:, :], sc[:, :], AF.Exp,
                                                     bias=kbias[:, kt:kt + 1],
                                                     scale=SCALE)
                                if limit >= 5:
                                    mm(ot[:, :],
                                       v_sb[kt][:, h * 65:(h + 1) * 65],
                                       pr[:, :], start=(kt == 0),
                                       stop=(kt == KT - 1))
                            if limit < 5:
                                continue
                            # normalize rows by the sums row (row 64): reciprocal
                            # of the PSUM row into SBUF, broadcast across 64
                            # partitions via a K=1 ones outer-product on PE
                            # (SBUF APs can't stride-0 partitions, DMA can't
                            # read PSUM), then multiply.
                            rc1 = den_pool.tile([1, QHW], F32, name="rc1", tag="rc1")
                            nc.vector.reciprocal(rc1[:, :], ot[64:65, :])
                            rb = psS.tile([64, QHW], F32, tag="psS")
                            for nch in range(2):
                                nc.tensor.matmul(
                                    rb[:, nch * 512:(nch + 1) * 512], ones64[:, :],
                                    rc1[:, nch * 512:(nch + 1) * 512],
                                    start=True, stop=True)
                            rec = den_pool.tile([64, QHW], F32, name="rec", tag="rec")
                            nc.vector.tensor_copy(rec[:, :], rb[:, :])
                            nc.vector.tensor_tensor(
                                opairs[pr_i][hh * 64:(hh + 1) * 64, :],
                                ot[0:64, :], rec[:, :], op=ALU.mult)

                        # output projection for this q half
                        if limit < 6:
                            continue
                        for q_t in range(QHW // 128):
                            for nch in range(2):
                                ps = psS.tile([128, 512], F32, tag="psS")
                                for pr_i in range(2):
                                    nc.tensor.matmul(
                                        ps[:, :],
                                        opairs[pr_i][:, q_t * 128:(q_t + 1) * 128],
                                        wo_sb[pr_i][:, nch * 512:(nch + 1) * 512],
                                        start=(pr_i == 0), stop=(pr_i == 1))
                                ost = ostage_pool.tile([128, 512], F32, name="ost",
                                                       tag="ost")
                                nc.vector.tensor_copy(ost[:, :], ps[:, :])
                                r0 = qh * QHW + q_t * 128
                                nc.sync.dma_start(
                                    out=out_d.ap()[r0:r0 + 128,
                                                   nch * 512:(nch + 1) * 512],
                                    in_=ost[:, :])

    nc.compile()
    return nc


def _get_program(use_attn_mask: bool, repeat: int = 1, limit: int = 99):
    key = (use_attn_mask, repeat, limit)
    if key not in _PROGRAM_CACHE:
        _PROGRAM_CACHE[key] = _build_program(use_attn_mask, repeat, limit)
    return _PROGRAM_CACHE[key]


def _prep_in_maps(query, key, value, key_padding_mask, attn_mask, stoich_frac,
                  Wq, bq, Wk, bk, Wv, bv, Wo, stoich_alpha, use_attn_mask):
    bf = np.float16
    f16 = np.float16
    alpha = float(stoich_alpha)
    # identity scale: 8*alpha normally; when alpha == 0 the stoich term is
    # removed by zeroing f instead, so the identity keeps scale 8 for the
    # (optional) attn_mask path.
    if alpha != 0.0:
        id_scale, am_scale = 8.0 * alpha, 1.0 / alpha
    else:
        id_scale, am_scale = 8.0, 1.0
        stoich_frac = np.zeros_like(stoich_frac)
    alpha2 = np.array([id_scale, 0.0], np.float32)
    xt = {}
    for b in range(B):
        xt["q", b] = np.ascontiguousarray(query[b].T).astype(bf)
        xt["k", b] = np.ascontiguousarray(key[b].T).astype(bf)
        xt["v", b] = np.ascontiguousarray(value[b].T).astype(bf)
    fqb, fcol, kb = {}, {}, {}
    for b in range(B):
        f32 = np.asarray(stoich_frac[b], np.float32)
        fqb[b] = np.ascontiguousarray(np.broadcast_to(f32, (128, T)))
        fcol[b] = np.ascontiguousarray(f32.reshape(KT, 128).T)
        kbv = -30000.0 * np.asarray(key_padding_mask[b], np.float32)
        kb[b] = np.ascontiguousarray(kbv.reshape(KT, 128).T)
    alc = np.ascontiguousarray(np.broadcast_to(alpha2, (128, 2)))
    wqT = np.ascontiguousarray(Wq.T).astype(bf)
    wkT = np.ascontiguousarray(Wk.T).astype(bf)
    wvT = np.ascontiguousarray(Wv.T).astype(bf)
    if use_attn_mask:
        # pre-divided by alpha: the scaled identity multiplies it back
        am8t = np.ascontiguousarray(am_scale * attn_mask.T).astype(f16)
    in_maps = []
    for c in range(N_CORES):
        b = c // 4
        g = c % 4
        sl = slice(g * DSL, (g + 1) * DSL)
        m = {
            "xq_t": xt["q", b],
            "xk_t": xt["k", b],
            "xv_t": xt["v", b],
            "wq_t": np.ascontiguousarray(wqT[:, sl]),
            "wk_t": np.ascontiguousarray(wkT[:, sl]),
            "wv_t": np.ascontiguousarray(wvT[:, sl]),
            "wo_t": np.ascontiguousarray(Wo[:, sl].T).astype(bf),
            "bqc_d": np.ascontiguousarray(
                np.asarray(bq[sl], np.float32).reshape(2, 128).T),
            "bkc_d": np.ascontiguousarray(
                np.asarray(bk[sl], np.float32).reshape(2, 128).T),
            "bvb_d": np.ascontiguousarray(np.broadcast_to(
                np.asarray(bv[sl], np.float32), (128, DSL))),
            "fqb_d": fqb[b],
            "fcol_d": fcol[b],
            "kb_d": kb[b],
            "alc_d": alc,
        }
        if use_attn_mask:
            m["am_d"] = am8t
        in_maps.append(m)
    return in_maps


def kernel(query, key, value, key_padding_mask, attn_mask, stoich_frac,
           Wq, bq, Wk, bk, Wv, bv, Wo, bo, stoich_alpha):
    query = np.asarray(query, np.float32)
    key = np.asarray(key, np.float32)
    value = np.asarray(value, np.float32)
    key_padding_mask = np.asarray(key_padding_mask)
    attn_mask = np.asarray(attn_mask, np.float32)
    stoich_frac = np.asarray(stoich_frac, np.float32)
    use_attn_mask = bool(np.any(attn_mask))

    nc = _get_program(use_attn_mask)
    in_maps = _prep_in_maps(query, key, value, key_padding_mask, attn_mask,
                            stoich_frac, Wq, bq, Wk, bk, Wv, bv, Wo,
                            stoich_alpha, use_attn_mask)
    res = run_bass_kernel_spmd(nc, in_maps, core_ids=list(range(N_CORES)))

    out = np.zeros((B, T, D_MODEL), np.float32)
    for c in range(N_CORES):
        out[c // 4] += res.results[c]["out_d"]
    out += np.asarray(bo, np.float32)[None, None, :]
    return out



# revision 14
# speedup vs baseline: 1.4748x; 1.4748x over previous
"""Trainium2 Bass kernel for nn_CustomMultiHeadAttention_20418274525443.

Self-contained: takes FULL unsharded inputs (as produced by the problem's
setup_inputs), shards across 8 NeuronCores, runs a Bass/Tile kernel via
run_bass_kernel_spmd, and gathers the full output.

Sharding: core c handles batch b = c//4 and heads 4*(c%4) .. 4*(c%4)+3
(data parallel on B x tensor parallel on heads). Each core computes its
partial output projection (contribution of its 256 hidden dims); the host
sums the 4 partials per batch and adds the output bias.

Stoichiometric bias via matmul-fused polynomial: the pairwise bias
  bias(f_q, f_k) = alpha * copysign(log1p|f_q - f_k|, f_q - f_k)
is a smooth odd function of d = f_q - f_k; we approximate alpha*g(d) by an
odd polynomial sum_k c_k d^k (degrees 1..9, max err ~2.6e-3 on g) and
expand in centered variables u = f_q - 1/2, v = f_k - 1/2:
  8*alpha*sum_k c_k (u-v)^k = sum_{j=0..9} uq_j(u) * vk_j(v)
The 10 rank terms become 10 extra contraction rows appended to each head's
Q^T/K^T tiles (head_dim 64 -> K=74 matmul): matmul cost is N-cycles
regardless of K<=128, so the bias costs ZERO extra PE cycles and removes
the per-head PSUM-preload matmuls, the per-tile log1p/sign vector chain,
and the Ln<->Exp activation-table switches of the previous version.
(The factor 8 pre-compensates the 0.125 softmax scale applied in exp.)

Math per core (Dh = 64, scale = 1/8):
  per head h: tiles qt_h/kt_h [128, T]: rows 0:64 = head's Q^T/K^T,
    rows 64:74 = basis uq/vk (host-computed from stoich_frac).
  scores^T[k,q] PSUM = kt_h[0:74].T @ qt_h[0:74]   (includes 8*bias)
  probs^T = Exp(0.125 * PSUM + kpm_bias_k) -> f16
  outext^T[65, q]: lhsT=[V_h|ones] so row 64 = sum_k probs (denominator)
  scaled^T = outext^T[0:64] * (1 / outext^T[64])
  out_partial[q, :] = concat_h(scaled^T).T @ Wo_slice^T  [2048, 1024] fp32
"""
import sys

sys.path.insert(0, "/opt/trn_rl_repo")

import numpy as np
from math import comb

import concourse.bass as bass
import concourse.tile as tile
from concourse import bacc, mybir
from concourse.bass_utils import run_bass_kernel_spmd
from concourse.masks import make_identity

D_MODEL = 1024
NHEAD = 16
HEAD_DIM = 64
B, T = 2, 2048
N_CORES = 8
HPC = 4               # heads per core
DSL = HPC * HEAD_DIM  # 256 = per-core slice of d_model
SCALE = HEAD_DIM ** -0.5  # 0.125
QH = 2                # q halves
QHW = T // QH         # 1024
KT = T // 128         # 16 k tiles
NB = 10               # stoich-bias basis rank (poly degrees 0..9)
KEXT = HEAD_DIM + NB  # 74 = extended contraction
F32 = mybir.dt.float32
F16 = mybir.dt.float16
BF16 = mybir.dt.bfloat16
AF = mybir.ActivationFunctionType
ALU = mybir.AluOpType

# odd-polynomial coefficients for g(d) = copysign(log1p|d|, d) on [-1, 1]
# (chebyshev-weighted least squares, degrees 1,3,5,7,9; max err 2.6e-3)
_POLY_DEGS = (1, 3, 5, 7, 9)

_PROGRAM_CACHE = {}
_POLY_CACHE = {}


def _poly_coeffs():
    if "c" not in _POLY_CACHE:
        d = np.linspace(-1, 1, 20001)
        g = np.sign(d) * np.log1p(np.abs(d))
        A = np.stack([d ** k for k in _POLY_DEGS], 1)
        w = 1.0 / np.sqrt(1 - np.clip(d, -0.9999, 0.9999) ** 2)
        _POLY_CACHE["c"] = np.linalg.lstsq(A * w[:, None], g * w, rcond=None)[0]
    return _POLY_CACHE["c"]


def _build_basis(f, alpha):
    """uq [NB, T] f16 (q-side), vk [NB, T] f16 (k-side):
    sum_j uq[j, q] * vk[j, k] == 8 * alpha * g_poly(f[q] - f[k])."""
    c = _poly_coeffs()
    u = np.asarray(f, np.float64) - 0.5
    uq = np.stack([u ** j for j in range(NB)], 0)
    vk = np.zeros((NB, u.shape[0]))
    for j in range(NB):
        for k, ck in zip(_POLY_DEGS, c):
            if k >= j:
                vk[j] += 8.0 * alpha * ck * comb(k, j) * ((-u) ** (k - j))
    for j in range(NB):  # balance magnitudes for f16
        su = np.abs(uq[j]).max()
        sv = np.abs(vk[j]).max()
        if su > 0 and sv > 0:
            s = np.sqrt(sv / su)
            uq[j] *= s
            vk[j] /= s
    return uq.astype(np.float16), vk.astype(np.float16)


def _build_program(use_attn_mask: bool, repeat: int = 1, limit: int = 99):
    # limit: 1=consts+weights, 2=+projections, 4=+scores/exp,
    # 5=+PV/normalize, 99=full
    nc = bacc.Bacc(num_devices=N_CORES)

    # ---- per-core DRAM inputs (host pre-sliced / transposed / cast) ----
    xq_t = nc.dram_tensor("xq_t", [D_MODEL, T], F16, kind="ExternalInput")
    xk_t = nc.dram_tensor("xk_t", [D_MODEL, T], F16, kind="ExternalInput")
    xv_t = nc.dram_tensor("xv_t", [D_MODEL, T], F16, kind="ExternalInput")
    wq_t = nc.dram_tensor("wq_t", [D_MODEL, DSL], F16, kind="ExternalInput")
    wk_t = nc.dram_tensor("wk_t", [D_MODEL, DSL], F16, kind="ExternalInput")
    wv_t = nc.dram_tensor("wv_t", [D_MODEL, DSL], F16, kind="ExternalInput")
    wo_t = nc.dram_tensor("wo_t", [DSL, D_MODEL], F16, kind="ExternalInput")
    bqc_d = nc.dram_tensor("bqc_d", [128, 2], F32, kind="ExternalInput")
    bkc_d = nc.dram_tensor("bkc_d", [128, 2], F32, kind="ExternalInput")
    bvb_d = nc.dram_tensor("bvb_d", [128, DSL], F32, kind="ExternalInput")
    kb_d = nc.dram_tensor("kb_d", [128, KT], F32, kind="ExternalInput")
    uq_d = nc.dram_tensor("uq_d", [NB, T], F16, kind="ExternalInput")
    vk_d = nc.dram_tensor("vk_d", [NB, T], F16, kind="ExternalInput")
    if use_attn_mask:
        am_d = nc.dram_tensor("am_d", [T, T], F16, kind="ExternalInput")
    out_d = nc.dram_tensor("out_d", [T, D_MODEL], F16, kind="ExternalOutput")

    def mm(out_ap, lhsT, rhs, start, stop, nmax=512):
        # matmul with the free dim chunked to one PSUM bank (<=512 fp32)
        n = rhs.shape[-1]
        assert out_ap.shape[-1] == n
        for c0 in range(0, n, nmax):
            c = slice(c0, min(c0 + nmax, n))
            nc.tensor.matmul(out_ap[:, c], lhsT, rhs[:, c],
                             start=start, stop=stop)

    with tile.TileContext(nc) as tc:
        import contextlib
        with contextlib.ExitStack() as ctx:
            const = ctx.enter_context(tc.tile_pool(name="const", bufs=1))
            qk_pool = ctx.enter_context(tc.tile_pool(name="qk", bufs=1))
            v_pool = ctx.enter_context(tc.tile_pool(name="vsb", bufs=1))
            w2_pool = ctx.enter_context(tc.tile_pool(name="wo", bufs=1))
            probs_pool = ctx.enter_context(tc.tile_pool(name="probs", bufs=4))
            den_pool = ctx.enter_context(tc.tile_pool(name="den", bufs=2))
            opair_pool = ctx.enter_context(tc.tile_pool(name="opair", bufs=4))
            ostage_pool = ctx.enter_context(tc.tile_pool(name="ostage", bufs=2))

            # ---- weights: one batched strided DMA per tensor; block di
            # lives at cols [di*DSL, (di+1)*DSL) ----
            wbig = {}
            for nm, dram in (("q", wq_t), ("k", wk_t), ("v", wv_t)):
                t_ = const.tile([128, 8 * DSL], F16, name=f"w{nm}", tag=f"w{nm}")
                nc.sync.dma_start(
                    out=t_[:, :].rearrange("p (a d) -> p a d", a=8),
                    in_=dram.ap().rearrange("(a p) d -> p a d", p=128))
                wbig[nm] = t_

            def wsl(nm, di, c0, c1):
                return wbig[nm][:, di * DSL + c0:di * DSL + c1]

            # ---- constants (Act hwdge queue; SP queue stays free for x) ----
            ones64 = const.tile([1, 64], F32)
            nc.vector.memset(ones64[:, :], 1.0)
            kbias = const.tile([128, KT], F32)
            nc.scalar.dma_start(out=kbias[:, :], in_=kb_d.ap())
            bq_col = const.tile([128, 2], F32)
            nc.scalar.dma_start(out=bq_col[:, :], in_=bqc_d.ap())
            bk_col = const.tile([128, 2], F32)
            nc.scalar.dma_start(out=bk_col[:, :], in_=bkc_d.ap())
            bv_bc = const.tile([128, DSL], F32)
            nc.scalar.dma_start(out=bv_bc[:, :], in_=bvb_d.ap())
            if use_attn_mask:
                ident_f = const.tile([128, 128], F32)
                make_identity(nc, ident_f[:, :])
                ident8 = const.tile([128, 128], F16)
                nc.vector.tensor_scalar(ident8[:, :], ident_f[:, :], 8.0, None,
                                        op0=ALU.mult)
            wo_sb = []
            for pr in range(2):
                t_ = w2_pool.tile([128, D_MODEL], F16, name=f"wo{pr}")
                nc.scalar.dma_start(out=t_[:, :],
                                    in_=wo_t.ap()[pr * 128:(pr + 1) * 128, :])
                wo_sb.append(t_)

            if use_attn_mask:
                am_sb = []
                for kt in range(KT):
                    t_ = const.tile([128, T], F16, name=f"am{kt}", tag=f"am{kt}")
                    # host passes attn_mask^T, [k, q] orientation (x8 via ident8)
                    nc.scalar.dma_start(out=t_[:, :],
                                        in_=am_d.ap()[kt * 128:(kt + 1) * 128, :])
                    am_sb.append(t_)

            # ---- per-head extended Q/K tiles + V tiles (buffers persist
            # across reps; data rows rewritten per rep, basis rows and the
            # ones-columns of V written once) ----
            qt_sb, kt_sb = [], []
            for h in range(HPC):
                qt_ = qk_pool.tile([128, T], F16, name=f"qth{h}")
                nc.scalar.dma_start(out=qt_[HEAD_DIM:KEXT, :], in_=uq_d.ap())
                qt_sb.append(qt_)
                kt_ = qk_pool.tile([128, T], F16, name=f"kth{h}")
                nc.scalar.dma_start(out=kt_[HEAD_DIM:KEXT, :], in_=vk_d.ap())
                kt_sb.append(kt_)
            v_sb = []
            for kt in range(KT):
                t_ = v_pool.tile([128, HPC * 65], F16, name=f"v{kt}")
                nc.vector.memset(t_[:, :], 1.0)  # ones columns survive at 65h+64
                v_sb.append(t_)

            for _rep in range(repeat):
                # ---- phase 1: projections ----
                if limit < 2:
                    continue
                with tc.tile_pool(name="xt", bufs=24) as xt_pool:
                    # all x DMAs up front on the SP queue: they stream in
                    # arrival order while the PE works through projections
                    x_t = {}
                    for nm, xdram in (("q", xq_t), ("k", xk_t), ("v", xv_t)):
                        for di in range(8):
                            xt_ = xt_pool.tile([128, T], F16, name=f"x{nm}{di}",
                                               tag="xt")
                            eng = nc.sync if di % 2 == 0 else nc.scalar
                            eng.dma_start(
                                out=xt_[:, :],
                                in_=xdram.ap()[di * 128:(di + 1) * 128, :])
                            x_t[nm, di] = xt_
                    # Q/K projections, di-outer so compute starts on the
                    # first x tile instead of the last
                    with tc.tile_pool(name="psA", bufs=4, space="PSUM") as psA:
                        for nm, bias_col, outs in (("q", bq_col, qt_sb),
                                                   ("k", bk_col, kt_sb)):
                            grp = {}
                            for do_t in range(2):
                                for nch in range(2):
                                    grp[do_t, nch] = psA.tile(
                                        [128, QHW], F32,
                                        name=f"psA{do_t}{nch}", tag="psA")
                            for di in range(8):
                                for do_t in range(2):
                                    for nch in range(2):
                                        mm(grp[do_t, nch][:, :],
                                           wsl(nm, di, do_t * 128,
                                               (do_t + 1) * 128),
                                           x_t[nm, di][:, nch * QHW:
                                                       (nch + 1) * QHW],
                                           start=(di == 0), stop=(di == 7))
                            # PSUM -> per-head SBUF f16 with bias add; psum
                            # rows 64:128 shift to head-tile rows 0:64
                            for do_t in range(2):
                                for nch in range(2):
                                    nsl = slice(nch * QHW, (nch + 1) * QHW)
                                    for hh in range(2):
                                        h = 2 * do_t + hh
                                        nc.vector.tensor_scalar(
                                            outs[h][0:HEAD_DIM, nsl],
                                            grp[do_t, nch][hh * 64:
                                                           (hh + 1) * 64, :],
                                            bias_col[hh * 64:(hh + 1) * 64,
                                                     do_t:do_t + 1],
                                            None, op0=ALU.add)
                    # V projection (natural layout), di-inner: one PSUM
                    # accumulation group per bank at a time
                    with tc.tile_pool(name="psV", bufs=2, space="PSUM") as psV:
                        for tt in range(KT):
                            ps = psV.tile([128, 2 * DSL], F32, name="psv",
                                          tag="psV")
                            for di in range(8):
                                mm(ps[:, 0:DSL],
                                   x_t["v", di][:, tt * 128:(tt + 1) * 128],
                                   wsl("v", di, 0, DSL),
                                   start=(di == 0), stop=(di == 7))
                            # strided copy into cols h*65..h*65+63 with bv
                            # add; ones columns at h*65+64 from the memset
                            vdst = v_sb[tt][:, :].rearrange(
                                "p (h e) -> p h e", e=65)[:, :, 0:HEAD_DIM]
                            nc.vector.tensor_tensor(
                                vdst,
                                ps[:, 0:DSL].rearrange("p (h e) -> p h e",
                                                       e=HEAD_DIM),
                                bv_bc[:, :].rearrange("p (h e) -> p h e",
                                                      e=HEAD_DIM),
                                op=ALU.add)

                # ---- phase 2: attention + output proj, per q-half ----
                # PSUM budget (8 banks): psS 2 x [128,1024] = 4 banks,
                # psO 2 x [65,1024] = 4 banks.  Heads processed in pairs,
                # interleaved per k-tile so the Act engine (exp) never
                # starves; PV lags QK/exp by one k-tile.
                with tc.tile_pool(name="psS", bufs=2, space="PSUM") as psS, \
                     tc.tile_pool(name="psO", bufs=2, space="PSUM") as psO:
                    if limit < 4:
                        continue
                    from collections import deque
                    pending = deque()

                    def drain(n=1):
                        for _ in range(n):
                            if pending:
                                pending.popleft()()

                    def chain_emit(ot, op_t, hh):
                        # normalize rows by the sums row (row 64)
                        rc1 = den_pool.tile([1, QHW], F32, name="rc1",
                                            tag="rc1")
                        nc.vector.reciprocal(rc1[:, :], ot[64:65, :])
                        rb = psS.tile([64, QHW], F32, tag="psS")
                        for nch2 in range(2):
                            nc.tensor.matmul(
                                rb[:, nch2 * 512:(nch2 + 1) * 512],
                                ones64[:, :],
                                rc1[:, nch2 * 512:(nch2 + 1) * 512],
                                start=True, stop=True)
                        rec = den_pool.tile([64, QHW], F32, name="rec",
                                            tag="rec")
                        nc.vector.tensor_copy(rec[:, :], rb[:, :])
                        nc.vector.tensor_tensor(
                            op_t[hh * 64:(hh + 1) * 64, :],
                            ot[0:64, :], rec[:, :], op=ALU.mult)

                    megas = {}

                    def oproj_emit(qh, q_t):
                        # output projection for column-tile q_t of q-half qh,
                        # staged into the q-half's mega tile
                        if qh not in megas:
                            megas[qh] = ostage_pool.tile(
                                [128, 8 * D_MODEL], F16, name=f"mega{qh}",
                                tag="mega")
                        mega = megas[qh]
                        for nch in range(2):
                            ps = psS.tile([128, 512], F32, tag="psS")
                            for pr_i in range(2):
                                nc.tensor.matmul(
                                    ps[:, :],
                                    opair_qh[qh][pr_i][:,
                                                       q_t * 128:(q_t + 1) * 128],
                                    wo_sb[pr_i][:, nch * 512:(nch + 1) * 512],
                                    start=(pr_i == 0), stop=(pr_i == 1))
                            nc.vector.tensor_copy(
                                mega[:, q_t * D_MODEL + nch * 512:
                                     q_t * D_MODEL + (nch + 1) * 512],
                                ps[:, :])

                    def oproj_flush(qh):
                        # one strided DMA for the whole q-half
                        nc.sync.dma_start(
                            out=out_d.ap()[qh * QHW:(qh + 1) * QHW, :]
                                .rearrange("(a p) d -> p a d", p=128),
                            in_=megas[qh][:, :]
                                .rearrange("p (a d) -> p a d", a=8))

                    opair_qh = {}
                    for qh in range(QH):
                        qsl = slice(qh * QHW, (qh + 1) * QHW)
                        opairs = []
                        opair_qh[qh] = opairs
                        for pr_i in range(2):
                            hA, hB = 2 * pr_i, 2 * pr_i + 1
                            op_t = opair_pool.tile([128, QHW], F16,
                                                   name=f"opair{qh}_{pr_i}",
                                                   tag="opair")
                            opairs.append(op_t)
                            otA = psO.tile([65, QHW], F32, tag="psO")
                            otB = psO.tile([65, QHW], F32, tag="psO")
                            prs = {}
                            for kt in range(KT):
                                for h, ot in ((hA, otA), (hB, otB)):
                                    sc = psS.tile([128, QHW], F32, tag="psS")
                                    mm(sc[:, :],
                                       kt_sb[h][0:KEXT,
                                                kt * 128:(kt + 1) * 128],
                                       qt_sb[h][0:KEXT, qsl],
                                       start=True, stop=not use_attn_mask)
                                    if use_attn_mask:
                                        mm(sc[:, :], ident8[:, :],
                                           am_sb[kt][:, qsl],
                                           start=False, stop=True)
                                    pr = probs_pool.tile([128, QHW], F16,
                                                         name="pr", tag="pr")
                                    nc.scalar.activation(
                                        pr[:, :], sc[:, :], AF.Exp,
                                        bias=kbias[:, kt:kt + 1], scale=SCALE)
                                    prs[h, kt] = pr
                                if limit >= 5 and kt > 0:
                                    for h, ot in ((hA, otA), (hB, otB)):
                                        mm(ot[:, :],
                                           v_sb[kt - 1][:,
                                                        h * 65:(h + 1) * 65],
                                           prs[h, kt - 1][:, :],
                                           start=(kt == 1), stop=False)
                                        del prs[h, kt - 1]
                                # drain one deferred denominator-chain /
                                # O-proj item every other k-tile iteration
                                if kt % 2 == 1:
                                    drain(1)
                            if limit < 5:
                                continue
                            for h, ot in ((hA, otA), (hB, otB)):
                                mm(ot[:, :],
                                   v_sb[KT - 1][:, h * 65:(h + 1) * 65],
                                   prs[h, KT - 1][:, :],
                                   start=False, stop=True)
                            pending.append(
                                lambda ot=otA, op=op_t: chain_emit(ot, op, 0))
                            pending.append(
                                lambda ot=otB, op=op_t: chain_emit(ot, op, 1))
                        if limit >= 6:
                            for q_t in range(QHW // 128):
                                pending.append(
                                    lambda qh=qh, q_t=q_t: oproj_emit(qh, q_t))
                            pending.append(lambda qh=qh: oproj_flush(qh))
                    drain(len(pending))

    nc.compile()
    return nc


def _get_program(use_attn_mask: bool, repeat: int = 1, limit: int = 99):
    key = (use_attn_mask, repeat, limit)
    if key not in _PROGRAM_CACHE:
        _PROGRAM_CACHE[key] = _build_program(use_attn_mask, repeat, limit)
    return _PROGRAM_CACHE[key]


def _prep_in_maps(query, key, value, key_padding_mask, attn_mask, stoich_frac,
                  Wq, bq, Wk, bk, Wv, bv, Wo, stoich_alpha, use_attn_mask):
    bf = np.float16
    f16 = np.float16
    alpha = float(stoich_alpha)
    xt = {}
    for b in range(B):
        xt["q", b] = np.ascontiguousarray(query[b].T).astype(bf)
        xt["k", b] = np.ascontiguousarray(key[b].T).astype(bf)
        xt["v", b] = np.ascontiguousarray(value[b].T).astype(bf)
    uqs, vks, kb = {}, {}, {}
    for b in range(B):
        f32 = np.asarray(stoich_frac[b], np.float32)
        uqs[b], vks[b] = _build_basis(f32, alpha)
        kbv = -30000.0 * np.asarray(key_padding_mask[b], np.float32)
        kb[b] = np.ascontiguousarray(kbv.reshape(KT, 128).T)
    wqT = np.ascontiguousarray(Wq.T).astype(bf)
    wkT = np.ascontiguousarray(Wk.T).astype(bf)
    wvT = np.ascontiguousarray(Wv.T).astype(bf)
    if use_attn_mask:
        am8t = np.ascontiguousarray(attn_mask.T).astype(f16)
    in_maps = []
    for c in range(N_CORES):
        b = c // 4
        g = c % 4
        sl = slice(g * DSL, (g + 1) * DSL)
        m = {
            "xq_t": xt["q", b],
            "xk_t": xt["k", b],
            "xv_t": xt["v", b],
            "wq_t": np.ascontiguousarray(wqT[:, sl]),
            "wk_t": np.ascontiguousarray(wkT[:, sl]),
            "wv_t": np.ascontiguousarray(wvT[:, sl]),
            "wo_t": np.ascontiguousarray(Wo[:, sl].T).astype(bf),
            "bqc_d": np.ascontiguousarray(
                np.asarray(bq[sl], np.float32).reshape(2, 128).T),
            "bkc_d": np.ascontiguousarray(
                np.asarray(bk[sl], np.float32).reshape(2, 128).T),
            "bvb_d": np.ascontiguousarray(np.broadcast_to(
                np.asarray(bv[sl], np.float32), (128, DSL))),
            "kb_d": kb[b],
            "uq_d": uqs[b],
            "vk_d": vks[b],
        }
        if use_attn_mask:
            m["am_d"] = am8t
        in_maps.append(m)
    return in_maps


def kernel(query, key, value, key_padding_mask, attn_mask, stoich_frac,
           Wq, bq, Wk, bk, Wv, bv, Wo, bo, stoich_alpha):
    query = np.asarray(query, np.float32)
    key = np.asarray(key, np.float32)
    value = np.asarray(value, np.float32)
    key_padding_mask = np.asarray(key_padding_mask)
    attn_mask = np.asarray(attn_mask, np.float32)
    stoich_frac = np.asarray(stoich_frac, np.float32)
    use_attn_mask = bool(np.any(attn_mask))

    nc = _get_program(use_attn_mask)
    in_maps = _prep_in_maps(query, key, value, key_padding_mask, attn_mask,
                            stoich_frac, Wq, bq, Wk, bk, Wv, bv, Wo,
                            stoich_alpha, use_attn_mask)
    res = run_bass_kernel_spmd(nc, in_maps, core_ids=list(range(N_CORES)))

    out = np.zeros((B, T, D_MODEL), np.float32)
    for c in range(N_CORES):
        out[c // 4] += np.asarray(res.results[c]["out_d"], np.float32)
    out += np.asarray(bo, np.float32)[None, None, :]
    return out


# revision 19
# speedup vs baseline: 2.1531x; 1.4600x over previous
"""Trainium2 Bass kernel for nn_CustomMultiHeadAttention_20418274525443.

Self-contained: takes FULL unsharded inputs (as produced by the problem's
setup_inputs), shards across 8 NeuronCores, runs a Bass/Tile kernel via
run_bass_kernel_spmd, and gathers the full output.

Sharding: core c handles batch b = c//4 and heads 4*(c%4) .. 4*(c%4)+3
(data parallel on B x tensor parallel on heads). Each core computes its
partial output projection (contribution of its 256 hidden dims); the host
sums the 4 partials per batch and adds the output bias.

Stoichiometric bias via matmul-fused polynomial: the pairwise bias
  bias(f_q, f_k) = alpha * copysign(log1p|f_q - f_k|, f_q - f_k)
is a smooth odd function of d = f_q - f_k; we approximate alpha*g(d) by an
odd polynomial sum_k c_k d^k (degrees 1..9, max err ~2.6e-3 on g) and
expand in centered variables u = f_q - 1/2, v = f_k - 1/2:
  8*alpha*sum_k c_k (u-v)^k = sum_{j=0..9} uq_j(u) * vk_j(v)
The 10 rank terms become 10 extra contraction rows appended to each head's
Q^T/K^T tiles (head_dim 64 -> K=74 matmul): matmul cost is N-cycles
regardless of K<=128, so the bias costs ZERO extra PE cycles and removes
the per-head PSUM-preload matmuls, the per-tile log1p/sign vector chain,
and the Ln<->Exp activation-table switches of the previous version.
(The factor 8 pre-compensates the 0.125 softmax scale applied in exp.)

Math per core (Dh = 64, scale = 1/8):
  per head h: tiles qt_h/kt_h [128, T]: rows 0:64 = head's Q^T/K^T,
    rows 64:74 = basis uq/vk (host-computed from stoich_frac).
  scores^T[k,q] PSUM = kt_h[0:74].T @ qt_h[0:74]   (includes 8*bias)
  probs^T = Exp(0.125 * PSUM + kpm_bias_k) -> f16
  outext^T[65, q]: lhsT=[V_h|ones] so row 64 = sum_k probs (denominator)
  scaled^T = outext^T[0:64] * (1 / outext^T[64])
  out_partial[q, :] = concat_h(scaled^T).T @ Wo_slice^T  [2048, 1024] fp32
"""
import sys

sys.path.insert(0, "/opt/trn_rl_repo")

import numpy as np
from math import comb

import concourse.bass as bass
import concourse.tile as tile
from concourse import bacc, mybir
from concourse.bass_utils import run_bass_kernel_spmd
from concourse.masks import make_identity

D_MODEL = 1024
NHEAD = 16
HEAD_DIM = 64
B, T = 2, 2048
N_CORES = 8
HPC = 4               # heads per core
DSL = HPC * HEAD_DIM  # 256 = per-core slice of d_model
SCALE = HEAD_DIM ** -0.5  # 0.125
QH = 2                # q halves
QHW = T // QH         # 1024
KT = T // 128         # 16 k tiles
NB = 10               # stoich-bias basis rank (poly degrees 0..9)
KEXT = HEAD_DIM + NB  # 74 = extended contraction
F32 = mybir.dt.float32
F16 = mybir.dt.float16
BF16 = mybir.dt.bfloat16
AF = mybir.ActivationFunctionType
ALU = mybir.AluOpType

# odd-polynomial coefficients for g(d) = copysign(log1p|d|, d) on [-1, 1]
# (chebyshev-weighted least squares, degrees 1,3,5,7,9; max err 2.6e-3)
_POLY_DEGS = (1, 3, 5, 7, 9)

_PROGRAM_CACHE = {}
_POLY_CACHE = {}


def _poly_coeffs():
    if "c" not in _POLY_CACHE:
        d = np.linspace(-1, 1, 20001)
        g = np.sign(d) * np.log1p(np.abs(d))
        A = np.stack([d ** k for k in _POLY_DEGS], 1)
        w = 1.0 / np.sqrt(1 - np.clip(d, -0.9999, 0.9999) ** 2)
        _POLY_CACHE["c"] = np.linalg.lstsq(A * w[:, None], g * w, rcond=None)[0]
    return _POLY_CACHE["c"]


def _build_basis(f, alpha):
    """uq [NB, T] f16 (q-side), vk [NB, T] f16 (k-side):
    sum_j uq[j, q] * vk[j, k] == 8 * alpha * g_poly(f[q] - f[k])."""
    c = _poly_coeffs()
    u = np.asarray(f, np.float64) - 0.5
    uq = np.stack([u ** j for j in range(NB)], 0)
    vk = np.zeros((NB, u.shape[0]))
    for j in range(NB):
        for k, ck in zip(_POLY_DEGS, c):
            if k >= j:
                vk[j] += 8.0 * alpha * ck * comb(k, j) * ((-u) ** (k - j))
    for j in range(NB):  # balance magnitudes for f16
        su = np.abs(uq[j]).max()
        sv = np.abs(vk[j]).max()
        if su > 0 and sv > 0:
            s = np.sqrt(sv / su)
            uq[j] *= s
            vk[j] /= s
    return uq.astype(np.float16), vk.astype(np.float16)


def _build_program(use_attn_mask: bool, repeat: int = 1, limit: int = 99):
    # limit: 1=consts+weights, 2=+projections, 4=+scores/exp,
    # 5=+PV/normalize, 99=full
    nc = bacc.Bacc(num_devices=N_CORES)

    # ---- per-core DRAM inputs (host pre-sliced / transposed / cast) ----
    xq_t = nc.dram_tensor("xq_t", [D_MODEL, T], F16, kind="ExternalInput")
    xk_t = nc.dram_tensor("xk_t", [D_MODEL, T], F16, kind="ExternalInput")
    xv_t = nc.dram_tensor("xv_t", [D_MODEL, T], F16, kind="ExternalInput")
    wq_t = nc.dram_tensor("wq_t", [D_MODEL, DSL], F16, kind="ExternalInput")
    wk_t = nc.dram_tensor("wk_t", [D_MODEL, DSL], F16, kind="ExternalInput")
    wv_t = nc.dram_tensor("wv_t", [D_MODEL, DSL], F16, kind="ExternalInput")
    wo_t = nc.dram_tensor("wo_t", [DSL, D_MODEL], F16, kind="ExternalInput")
    bqc_d = nc.dram_tensor("bqc_d", [128, 2], F32, kind="ExternalInput")
    bkc_d = nc.dram_tensor("bkc_d", [128, 2], F32, kind="ExternalInput")
    bvb_d = nc.dram_tensor("bvb_d", [128, DSL], F32, kind="ExternalInput")
    kb_d = nc.dram_tensor("kb_d", [128, KT], F32, kind="ExternalInput")
    uq_d = nc.dram_tensor("uq_d", [NB, T], F16, kind="ExternalInput")
    vk_d = nc.dram_tensor("vk_d", [NB, T], F16, kind="ExternalInput")
    if use_attn_mask:
        am_d = nc.dram_tensor("am_d", [T, T], F16, kind="ExternalInput")
    out_d = nc.dram_tensor("out_d", [T, D_MODEL], F16, kind="ExternalOutput")

    def mm(out_ap, lhsT, rhs, start, stop, nmax=512):
        # matmul with the free dim chunked to one PSUM bank (<=512 fp32)
        n = rhs.shape[-1]
        assert out_ap.shape[-1] == n
        for c0 in range(0, n, nmax):
            c = slice(c0, min(c0 + nmax, n))
            nc.tensor.matmul(out_ap[:, c], lhsT, rhs[:, c],
                             start=start, stop=stop)

    with tile.TileContext(nc) as tc:
        import contextlib
        with contextlib.ExitStack() as ctx:
            const = ctx.enter_context(tc.tile_pool(name="const", bufs=1))
            qk_pool = ctx.enter_context(tc.tile_pool(name="qk", bufs=1))
            v_pool = ctx.enter_context(tc.tile_pool(name="vsb", bufs=1))
            w2_pool = ctx.enter_context(tc.tile_pool(name="wo", bufs=1))
            probs_pool = ctx.enter_context(tc.tile_pool(name="probs", bufs=6))
            den_pool = ctx.enter_context(tc.tile_pool(name="den", bufs=2))
            opair_pool = ctx.enter_context(tc.tile_pool(name="opair", bufs=4))
            ostage_pool = ctx.enter_context(tc.tile_pool(name="ostage", bufs=1))

            # ---- weights: one batched strided DMA per tensor; block di
            # lives at cols [di*DSL, (di+1)*DSL) ----
            wbig = {}
            for nm, dram in (("q", wq_t), ("k", wk_t), ("v", wv_t)):
                t_ = const.tile([128, 8 * DSL], F16, name=f"w{nm}", tag=f"w{nm}")
                nc.sync.dma_start(
                    out=t_[:, :].rearrange("p (a d) -> p a d", a=8),
                    in_=dram.ap().rearrange("(a p) d -> p a d", p=128))
                wbig[nm] = t_

            def wsl(nm, di, c0, c1):
                return wbig[nm][:, di * DSL + c0:di * DSL + c1]

            # ---- constants (Act hwdge queue; SP queue stays free for x) ----
            ones64 = const.tile([1, 64], F32)
            nc.vector.memset(ones64[:, :], 1.0)
            kbias = const.tile([128, KT], F32)
            nc.scalar.dma_start(out=kbias[:, :], in_=kb_d.ap())
            bq_col = const.tile([128, 2], F32)
            nc.scalar.dma_start(out=bq_col[:, :], in_=bqc_d.ap())
            bk_col = const.tile([128, 2], F32)
            nc.scalar.dma_start(out=bk_col[:, :], in_=bkc_d.ap())
            bv_bc = const.tile([128, DSL], F32)
            nc.scalar.dma_start(out=bv_bc[:, :], in_=bvb_d.ap())
            if use_attn_mask:
                ident_f = const.tile([128, 128], F32)
                make_identity(nc, ident_f[:, :])
                ident8 = const.tile([128, 128], F16)
                nc.vector.tensor_scalar(ident8[:, :], ident_f[:, :], 8.0, None,
                                        op0=ALU.mult)
            wo_sb = []
            for pr in range(2):
                t_ = w2_pool.tile([128, D_MODEL], F16, name=f"wo{pr}")
                nc.scalar.dma_start(out=t_[:, :],
                                    in_=wo_t.ap()[pr * 128:(pr + 1) * 128, :])
                wo_sb.append(t_)

            if use_attn_mask:
                am_sb = []
                for kt in range(KT):
                    t_ = const.tile([128, T], F16, name=f"am{kt}", tag=f"am{kt}")
                    # host passes attn_mask^T, [k, q] orientation (x8 via ident8)
                    nc.scalar.dma_start(out=t_[:, :],
                                        in_=am_d.ap()[kt * 128:(kt + 1) * 128, :])
                    am_sb.append(t_)

            # ---- per-head extended Q/K tiles + V tiles (buffers persist
            # across reps; data rows rewritten per rep, basis rows and the
            # ones-columns of V written once) ----
            qt_sb, kt_sb = [], []
            for h in range(HPC):
                qt_ = qk_pool.tile([128, T], F16, name=f"qth{h}")
                nc.scalar.dma_start(out=qt_[HEAD_DIM:KEXT, :], in_=uq_d.ap())
                qt_sb.append(qt_)
                kt_ = qk_pool.tile([128, T], F16, name=f"kth{h}")
                nc.scalar.dma_start(out=kt_[HEAD_DIM:KEXT, :], in_=vk_d.ap())
                kt_sb.append(kt_)
            v_sb = []
            for kt in range(KT):
                t_ = v_pool.tile([128, HPC * 65], F16, name=f"v{kt}")
                nc.vector.memset(t_[:, :], 1.0)  # ones columns survive at 65h+64
                v_sb.append(t_)

            for _rep in range(repeat):
                # ---- phase 1: projections ----
                if limit < 2:
                    continue
                xt_ctx = tc.tile_pool(name="xt", bufs=16)
                xt_pool = xt_ctx.__enter__()
                try:
                    # all x DMAs up front, alternating the two hwdge queues;
                    # they stream in arrival order while the PE computes
                    x_t = {}
                    for nm, xdram in (("q", xq_t), ("k", xk_t), ("v", xv_t)):
                        for di in range(8):
                            xt_ = xt_pool.tile([128, T], F16, name=f"x{nm}{di}",
                                               tag="xt")
                            eng = nc.sync if di % 2 == 0 else nc.scalar
                            eng.dma_start(
                                out=xt_[:, :],
                                in_=xdram.ap()[di * 128:(di + 1) * 128, :])
                            x_t[nm, di] = xt_
                    # Q/K projections: two di-outer passes of 2 groups each
                    # (psA = 4 PSUM banks, freed before phase 2 so the psS
                    # pool starts without waiting on V)
                    with tc.tile_pool(name="psA", bufs=2, space="PSUM") as psA:
                        for nm, bias_col, outs in (("q", bq_col, qt_sb),
                                                   ("k", bk_col, kt_sb)):
                            for do_t in range(2):
                                grp = {}
                                for nch in range(2):
                                    grp[nch] = psA.tile(
                                        [128, QHW], F32,
                                        name=f"psA{nch}", tag="psA")
                                for di in range(8):
                                    for nch in range(2):
                                        mm(grp[nch][:, :],
                                           wsl(nm, di, do_t * 128,
                                               (do_t + 1) * 128),
                                           x_t[nm, di][:, nch * QHW:
                                                       (nch + 1) * QHW],
                                           start=(di == 0), stop=(di == 7))
                                # PSUM -> per-head SBUF f16 with bias add;
                                # psum rows 64:128 shift to head-tile rows
                                # 0:64 (DVE); unshifted rows go via Act Copy
                                # (same act table as Exp, so no table load)
                                for nch in range(2):
                                    nsl = slice(nch * QHW, (nch + 1) * QHW)
                                    h0 = 2 * do_t
                                    nc.scalar.activation(
                                        outs[h0][0:HEAD_DIM, nsl],
                                        grp[nch][0:64, :], AF.Identity,
                                        bias=bias_col[0:64, do_t:do_t + 1],
                                        scale=1.0)
                                    nc.vector.tensor_scalar(
                                        outs[h0 + 1][0:HEAD_DIM, nsl],
                                        grp[nch][64:128, :],
                                        bias_col[64:128, do_t:do_t + 1],
                                        None, op0=ALU.add)

                    # ---- phase 2: attention + output proj, per q-half ----
                # PSUM budget (8 banks): psS 2 x [128,1024] = 4 banks,
                # psO 2 x [65,1024] = 4 banks.  Heads processed in pairs,
                # interleaved per k-tile so the Act engine (exp) never
                # starves; PV lags QK/exp by one k-tile.
                    if limit < 4:
                        continue
                    psS_ctx = tc.tile_pool(name="psS", bufs=2, space="PSUM")
                    psO_ctx = tc.tile_pool(name="psO", bufs=2, space="PSUM")
                    psS = psS_ctx.__enter__()
                    psO = psO_ctx.__enter__()
                    from collections import deque
                    pending = deque()

                    def drain(n=1):
                        for _ in range(n):
                            if pending:
                                pending.popleft()()

                    def v_emit(tt):
                        # V projection k-tile (natural layout), deferred into
                        # pair-0's loop; borrows a psS slot (one accumulation
                        # group in the slot's first bank)
                        ps = psS.tile([128, QHW], F32, name="psv", tag="psS")
                        for di in range(8):
                            mm(ps[:, 0:DSL],
                               x_t["v", di][:, tt * 128:(tt + 1) * 128],
                               wsl("v", di, 0, DSL),
                               start=(di == 0), stop=(di == 7))
                        # strided copy into cols h*65..h*65+63 with bv add;
                        # ones columns at h*65+64 from the memset
                        vdst = v_sb[tt][:, :].rearrange(
                            "p (h e) -> p h e", e=65)[:, :, 0:HEAD_DIM]
                        nc.vector.tensor_tensor(
                            vdst,
                            ps[:, 0:DSL].rearrange("p (h e) -> p h e",
                                                   e=HEAD_DIM),
                            bv_bc[:, :].rearrange("p (h e) -> p h e",
                                                  e=HEAD_DIM),
                            op=ALU.add)

                    for tt in range(KT):
                        pending.append(lambda tt=tt: v_emit(tt))

                    def chain_emit(ot, op_t, hh, tail=False):
                        # normalize rows by the sums row (row 64)
                        rc1 = den_pool.tile([1, QHW], F32, name="rc1",
                                            tag="rc1")
                        nc.vector.reciprocal(rc1[:, :], ot[64:65, :])
                        rb = psS.tile([64, QHW], F32, tag="psS")
                        for nch2 in range(2):
                            nc.tensor.matmul(
                                rb[:, nch2 * 512:(nch2 + 1) * 512],
                                ones64[:, :],
                                rc1[:, nch2 * 512:(nch2 + 1) * 512],
                                start=True, stop=True)
                        rec = den_pool.tile([64, QHW], F32, name="rec",
                                            tag="rec")
                        if tail:
                            nc.scalar.activation(rec[:, :], rb[:, :], AF.Copy,
                                                 bias=0.0, scale=1.0)
                        else:
                            nc.vector.tensor_copy(rec[:, :], rb[:, :])
                        nc.vector.tensor_tensor(
                            op_t[hh * 64:(hh + 1) * 64, :],
                            ot[0:64, :], rec[:, :], op=ALU.mult)

                    megas = {}

                    def oproj_emit(qh, q_t):
                        # output projection for column-tile q_t of q-half qh,
                        # staged into the q-half's mega tile
                        if qh not in megas:
                            megas[qh] = ostage_pool.tile(
                                [128, 8 * D_MODEL], F16, name=f"mega{qh}",
                                tag="mega")
                        mega = megas[qh]
                        for nch in range(2):
                            ps = psS.tile([128, 512], F32, tag="psS")
                            for pr_i in range(2):
                                nc.tensor.matmul(
                                    ps[:, :],
                                    opair_qh[qh][pr_i][:,
                                                       q_t * 128:(q_t + 1) * 128],
                                    wo_sb[pr_i][:, nch * 512:(nch + 1) * 512],
                                    start=(pr_i == 0), stop=(pr_i == 1))
                            dst = mega[:, q_t * D_MODEL + nch * 512:
                                       q_t * D_MODEL + (nch + 1) * 512]
                            if qh == 1:
                                nc.scalar.activation(dst, ps[:, :], AF.Copy,
                                                     bias=0.0, scale=1.0)
                            else:
                                nc.vector.tensor_copy(dst, ps[:, :])

                    def oproj_flush(qh):
                        # one strided DMA for the whole q-half
                        nc.sync.dma_start(
                            out=out_d.ap()[qh * QHW:(qh + 1) * QHW, :]
                                .rearrange("(a p) d -> p a d", p=128),
                            in_=megas[qh][:, :]
                                .rearrange("p (a d) -> p a d", a=8))

                    opair_qh = {}
                    for qh in range(QH):
                        qsl = slice(qh * QHW, (qh + 1) * QHW)
                        opairs = []
                        opair_qh[qh] = opairs
                        for pr_i in range(2):
                            hA, hB = 2 * pr_i, 2 * pr_i + 1
                            op_t = opair_pool.tile([128, QHW], F16,
                                                   name=f"opair{qh}_{pr_i}",
                                                   tag="opair")
                            opairs.append(op_t)
                            otA = psO.tile([65, QHW], F32, tag="psO")
                            otB = psO.tile([65, QHW], F32, tag="psO")
                            prs = {}
                            for kt in range(KT):
                                for h, ot in ((hA, otA), (hB, otB)):
                                    sc = psS.tile([128, QHW], F32, tag="psS")
                                    mm(sc[:, :],
                                       kt_sb[h][0:KEXT,
                                                kt * 128:(kt + 1) * 128],
                                       qt_sb[h][0:KEXT, qsl],
                                       start=True, stop=not use_attn_mask)
                                    if use_attn_mask:
                                        mm(sc[:, :], ident8[:, :],
                                           am_sb[kt][:, qsl],
                                           start=False, stop=True)
                                    pr = probs_pool.tile([128, QHW], F16,
                                                         name="pr", tag="pr")
                                    nc.scalar.activation(
                                        pr[:, :], sc[:, :], AF.Exp,
                                        bias=kbias[:, kt:kt + 1], scale=SCALE)
                                    prs[h, kt] = pr
                                if limit >= 5 and kt > 1:
                                    for h, ot in ((hA, otA), (hB, otB)):
                                        mm(ot[:, :],
                                           v_sb[kt - 2][:,
                                                        h * 65:(h + 1) * 65],
                                           prs[h, kt - 2][:, :],
                                           start=(kt == 2), stop=False)
                                        del prs[h, kt - 2]
                                # drain one deferred V-proj / denominator /
                                # O-proj item per k-tile iteration
                                drain(1)
                            if limit < 5:
                                continue
                            for ktt in (KT - 2, KT - 1):
                                for h, ot in ((hA, otA), (hB, otB)):
                                    mm(ot[:, :],
                                       v_sb[ktt][:, h * 65:(h + 1) * 65],
                                       prs[h, ktt][:, :],
                                       start=False, stop=(ktt == KT - 1))
                            last = (qh == 1 and pr_i == 1)
                            pending.append(
                                lambda ot=otA, op=op_t, tl=last:
                                chain_emit(ot, op, 0, tl))
                            pending.append(
                                lambda ot=otB, op=op_t, tl=last:
                                chain_emit(ot, op, 1, tl))
                        if limit >= 6:
                            for q_t in range(QHW // 128):
                                pending.append(
                                    lambda qh=qh, q_t=q_t: oproj_emit(qh, q_t))
                            pending.append(lambda qh=qh: oproj_flush(qh))
                    drain(len(pending))
                    psO_ctx.__exit__(None, None, None)
                    psS_ctx.__exit__(None, None, None)
                finally:
                    xt_ctx.__exit__(None, None, None)

    nc.compile()
    return nc


def _get_program(use_attn_mask: bool, repeat: int = 1, limit: int = 99):
    key = (use_attn_mask, repeat, limit)
    if key not in _PROGRAM_CACHE:
        _PROGRAM_CACHE[key] = _build_program(use_attn_mask, repeat, limit)
    return _PROGRAM_CACHE[key]


def _prep_in_maps(query, key, value, key_padding_mask, attn_mask, stoich_frac,
                  Wq, bq, Wk, bk, Wv, bv, Wo, stoich_alpha, use_attn_mask):
    bf = np.float16
    f16 = np.float16
    alpha = float(stoich_alpha)
    xt = {}
    for b in range(B):
        xt["q", b] = np.ascontiguousarray(query[b].T).astype(bf)
        xt["k", b] = np.ascontiguousarray(key[b].T).astype(bf)
        xt["v", b] = np.ascontiguousarray(value[b].T).astype(bf)
    uqs, vks, kb = {}, {}, {}
    for b in range(B):
        f32 = np.asarray(stoich_frac[b], np.float32)
        uqs[b], vks[b] = _build_basis(f32, alpha)
        kbv = -30000.0 * np.asarray(key_padding_mask[b], np.float32)
        kb[b] = np.ascontiguousarray(kbv.reshape(KT, 128).T)
    wqT = np.ascontiguousarray(Wq.T).astype(bf)
    wkT = np.ascontiguousarray(Wk.T).astype(bf)
    wvT = np.ascontiguousarray(Wv.T).astype(bf)
    if use_attn_mask:
        am8t = np.ascontiguousarray(attn_mask.T).astype(f16)
    in_maps = []
    for c in range(N_CORES):
        b = c // 4
        g = c % 4
        sl = slice(g * DSL, (g + 1) * DSL)
        m = {
            "xq_t": xt["q", b],
            "xk_t": xt["k", b],
            "xv_t": xt["v", b],
            "wq_t": np.ascontiguousarray(wqT[:, sl]),
            "wk_t": np.ascontiguousarray(wkT[:, sl]),
            "wv_t": np.ascontiguousarray(wvT[:, sl]),
            "wo_t": np.ascontiguousarray(Wo[:, sl].T).astype(bf),
            "bqc_d": np.ascontiguousarray(
                np.asarray(bq[sl], np.float32).reshape(2, 128).T),
            "bkc_d": np.ascontiguousarray(
                np.asarray(bk[sl], np.float32).reshape(2, 128).T),
            "bvb_d": np.ascontiguousarray(np.broadcast_to(
                np.asarray(bv[sl], np.float32), (128, DSL))),
            "kb_d": kb[b],
            "uq_d": uqs[b],
            "vk_d": vks[b],
        }
        if use_attn_mask:
            m["am_d"] = am8t
        in_maps.append(m)
    return in_maps


def kernel(query, key, value, key_padding_mask, attn_mask, stoich_frac,
           Wq, bq, Wk, bk, Wv, bv, Wo, bo, stoich_alpha):
    query = np.asarray(query, np.float32)
    key = np.asarray(key, np.float32)
    value = np.asarray(value, np.float32)
    key_padding_mask = np.asarray(key_padding_mask)
    attn_mask = np.asarray(attn_mask, np.float32)
    stoich_frac = np.asarray(stoich_frac, np.float32)
    use_attn_mask = bool(np.any(attn_mask))

    nc = _get_program(use_attn_mask)
    in_maps = _prep_in_maps(query, key, value, key_padding_mask, attn_mask,
                            stoich_frac, Wq, bq, Wk, bk, Wv, bv, Wo,
                            stoich_alpha, use_attn_mask)
    res = run_bass_kernel_spmd(nc, in_maps, core_ids=list(range(N_CORES)))

    out = np.zeros((B, T, D_MODEL), np.float32)
    for c in range(N_CORES):
        out[c // 4] += np.asarray(res.results[c]["out_d"], np.float32)
    out += np.asarray(bo, np.float32)[None, None, :]
    return out
